# revision 8
# baseline (speedup 1.0000x reference)
"""Causal self-attention (dense transformer block) on 8 Trainium2 NeuronCores.

Reference computation (per batch b of 4, 16 heads, head_dim 64, d=1024):
    qkv = x @ w_qkv + b_qkv ; q,k,v split
    att = softmax(causal(q k^T / 8)) ; y = att @ v ; out = y @ w_o + b_o

Sharding: batch (4) x head-halves (2) -> 8 cores. Core c handles batch c//2's
heads [8*(c%2), 8*(c%2)+8). Each core computes its QKV shard, causal attention
for its 8 heads, and a partial output projection over its 512 head-dims.
Host sums the two partials per batch and adds b_o.

On-core layout (all matmul operands float32r = full-rate fp32 on the PE):
  - x^T resident as [128, 8, 2048] per embed block (host pre-transposes).
  - Q^T, K^T per head-pair: [128, 2048] (head a on partitions 0:64, b on 64:128).
  - V per pair: [128(seq), kb, head, 65] with col 64 = 1.0 (softmax denominator
    rides along the att@V accumulation as output row 64).
  - scores computed transposed: s^T[k, q] = (K^T)^T Q^T, causally column-trimmed;
    triangular window masked by accumulating a -1e9 mask via identity matmul
    (keeps PSUM readers = ACT only).
  - exp on ScalarE with fused 1/8 scale; att@V accumulates y^T[65, q] per head;
    row 64 = sum -> reciprocal -> gpsimd partition-broadcast -> normalize.
  - output projection consumes y^T directly as lhsT.
"""

import numpy as np

D = 1024
SEQ = 2048
NH = 16
HD = 64
NCORES = 8
PAIRS = 4       # head-pairs per core (8 heads)
NQT = 4         # query tiles of 512
QT = 512
NKB = 16        # key blocks of 128
MASKVAL = -1e9

_CACHE = {}


def _build_nc():
    import concourse.mybir as mybir
    import concourse.tile as tile
    from concourse import bacc

    f32 = mybir.dt.float32
    f32r = mybir.dt.float32r
    EXP = mybir.ActivationFunctionType.Exp

    nc = bacc.Bacc("TRN2", target_bir_lowering=False, debug=False)
    xt_d = nc.dram_tensor("xt", [D, SEQ], f32r, kind="ExternalInput").ap()
    wqkv_d = nc.dram_tensor("wqkv", [D, 3 * 512], f32r, kind="ExternalInput").ap()
    bqkv_d = nc.dram_tensor("bqkv", [1, 3 * 512], f32r, kind="ExternalInput").ap()
    wo_d = nc.dram_tensor("wo", [512, D], f32r, kind="ExternalInput").ap()
    out_d = nc.dram_tensor("out", [SEQ, D], f32, kind="ExternalOutput").ap()

    with tile.TileContext(nc) as tc:
        with (
            tc.tile_pool(name="const", bufs=1) as constp,
            tc.tile_pool(name="qt", bufs=PAIRS) as qtp,
            tc.tile_pool(name="kt", bufs=PAIRS) as ktp,
            tc.tile_pool(name="v", bufs=PAIRS) as vp,
            tc.tile_pool(name="yt", bufs=PAIRS) as ytp,
        ):
            # fp32r constants; built on fp32 scratch (memset/affine_select
            # are not fp32r-legal) then rounded in via tensor_copy
            ones = constp.tile([1, 512], f32r)
            ident = constp.tile([128, 128], f32r)
            mask = constp.tile([128, QT], f32r)
            zeros = constp.tile([128, 384], f32r)
            col1 = constp.tile([128, 1], f32r)
            with tc.tile_pool(name="scratch", bufs=1) as scrp:
                ones32 = scrp.tile([1, 512], f32)
                nc.gpsimd.memset(ones32, 1.0)
                nc.vector.tensor_copy(ones, ones32)
                ident32 = scrp.tile([128, 128], f32)
                nc.gpsimd.memset(ident32, 0.0)
                nc.gpsimd.affine_select(
                    out=ident32, in_=ident32,
                    compare_op=mybir.AluOpType.not_equal,
                    fill=1.0, base=0, pattern=[[-1, 128]], channel_multiplier=1)
                nc.vector.tensor_copy(ident, ident32)
                # mask[k, q] = 0 if k <= q else MASKVAL for q < 128; 0 beyond
                # (zero tail so a full-width accumulate closes the PSUM bank)
                mask32 = scrp.tile([128, QT], f32)
                nc.gpsimd.memset(mask32, 0.0)
                nc.gpsimd.affine_select(
                    out=mask32[:, 0:128], in_=mask32[:, 0:128],
                    compare_op=mybir.AluOpType.is_ge,
                    fill=MASKVAL, base=0, pattern=[[1, 128]],
                    channel_multiplier=-1)
                nc.vector.tensor_copy(mask, mask32)
                zeros32 = scrp.tile([128, 384], f32)
                nc.gpsimd.memset(zeros32, 0.0)
                nc.vector.tensor_copy(zeros, zeros32)
                ones_col32 = scrp.tile([128, 1], f32)
                nc.gpsimd.memset(ones_col32, 1.0)
                nc.vector.tensor_copy(col1, ones_col32)

            qts = [qtp.tile([128, SEQ], f32r, tag="qt", name=f"qt{i}") for i in range(PAIRS)]
            kts = [ktp.tile([128, SEQ], f32r, tag="kt", name=f"kt{i}") for i in range(PAIRS)]
            vs = [vp.tile([128, NKB, 2, 65], f32r, tag="v", name=f"v{i}") for i in range(PAIRS)]
            yts = [ytp.tile([128, SEQ], f32r, tag="yt", name=f"yt{i}") for i in range(PAIRS)]
            for p in range(PAIRS):
                nc.vector.tensor_copy(
                    vs[p][:, :, :, 64:65],
                    col1[:, None, None, :].broadcast_to([128, NKB, 2, 1]))

            # ---- Phase 1: QKV projections (x^T streamed by seq-tile) ----
            with (
                tc.tile_pool(name="xt", bufs=2) as xtp,
                tc.tile_pool(name="w1", bufs=1) as w1p,
                tc.tile_pool(name="ps1", bufs=4, space="PSUM") as ps1,
            ):
                wqkv_sb = w1p.tile([128, 8, 1536], f32r)
                nc.sync.dma_start(
                    out=wqkv_sb, in_=wqkv_d.rearrange("(e p) m -> p e m", p=128))
                bqkv_sb = w1p.tile([1, 1536], f32r)
                nc.sync.dma_start(out=bqkv_sb, in_=bqkv_d)

                ST = 256  # xt streaming tile width (seq)
                for s in range(SEQ // ST):
                    xt_s = xtp.tile([128, 8, ST], f32r)
                    nc.sync.dma_start(
                        out=xt_s,
                        in_=xt_d.rearrange("(e p) s -> p e s", p=128)[
                            :, :, s * ST:(s + 1) * ST])
                    # Q^T and K^T slices for each pair
                    for p in range(PAIRS):
                        for qk in range(2):
                            cols = 512 * qk + 128 * p
                            ps = ps1.tile([128, ST], f32)
                            nc.tensor.matmul(
                                ps, bqkv_sb[0:1, cols:cols + 128], ones[0:1, :ST],
                                start=True, stop=False)
                            for e in range(8):
                                nc.tensor.matmul(
                                    ps, wqkv_sb[:, e, cols:cols + 128],
                                    xt_s[:, e, :], start=False, stop=(e == 7))
                            dst = qts[p] if qk == 0 else kts[p]
                            nc.vector.tensor_copy(
                                dst[:, s * ST:(s + 1) * ST], ps)
                    # V rows for this seq-tile (all 8 heads at once)
                    for s4 in range(ST // 128):
                        kb = (ST // 128) * s + s4
                        ps = ps1.tile([128, 512], f32, tag="psv")
                        nc.tensor.matmul(
                            ps, ones[0:1, 0:128], bqkv_sb[0:1, 1024:1536],
                            start=True, stop=False)
                        for e in range(8):
                            nc.tensor.matmul(
                                ps, xt_s[:, e, 128 * s4:128 * s4 + 128],
                                wqkv_sb[:, e, 1024:1536],
                                start=False, stop=(e == 7))
                        for p in range(PAIRS):
                            for h in range(2):
                                nc.vector.tensor_copy(
                                    vs[p][:, kb, h, 0:64],
                                    ps[:, 128 * p + 64 * h:128 * p + 64 * h + 64])

            # ---- Phase 2: causal attention per pair / query-tile ----
            with (
                tc.tile_pool(name="pr", bufs=6) as prp,
                tc.tile_pool(name="sums", bufs=4) as sup,
                tc.tile_pool(name="rbc", bufs=4) as rbp,
                tc.tile_pool(name="scps", bufs=4, space="PSUM") as scps,
                tc.tile_pool(name="ytps", bufs=4, space="PSUM") as ytps,
            ):
                for p in range(PAIRS):
                    for j in range(NQT):
                        q0 = j * QT
                        nkb = 4 * j + 4
                        yt_ps = [ytps.tile([65, QT], f32, tag="ytps", name=f"ytps{h}") for h in range(2)]
                        for kb in range(nkb):
                            d = max(0, 128 * kb - q0)
                            for h in range(2):
                                hs = 64 * h
                                sc = scps.tile([128, QT], f32)
                                diag = 128 * kb >= q0
                                nc.tensor.matmul(
                                    sc[:, d:QT],
                                    kts[p][hs:hs + 64, 128 * kb:128 * kb + 128],
                                    qts[p][hs:hs + 64, q0 + d:q0 + QT],
                                    start=True, stop=not diag)
                                if diag:
                                    nc.tensor.matmul(
                                        sc[:, d:QT], ident, mask[:, 0:QT - d],
                                        start=False, stop=True)
                                pr = prp.tile([128, QT], f32r)
                                if d:
                                    # fully-masked columns: probs = 0 so the
                                    # att@V matmul can run full width
                                    nc.vector.tensor_copy(
                                        pr[:, 0:d], zeros[:, 0:d])
                                nc.scalar.activation(
                                    out=pr[:, d:QT], in_=sc[:, d:QT],
                                    func=EXP, scale=0.125)
                                nc.tensor.matmul(
                                    yt_ps[h], vs[p][:, kb, h, :],
                                    pr, start=(kb == 0),
                                    stop=(kb == nkb - 1))
                        for h in range(2):
                            rc = sup.tile([1, QT], mybir.dt.float32)
                            nc.vector.reciprocal(rc, yt_ps[h][64:65, :])
                            rb = rbp.tile([64, QT], mybir.dt.float32)
                            nc.gpsimd.partition_broadcast(rb, rc)
                            nc.vector.tensor_mul(
                                yts[p][64 * h:64 * h + 64, q0:q0 + QT],
                                yt_ps[h][0:64, :], rb)

            # ---- Phase 3: output projection (partial over 512 head-dims) ----
            with (
                tc.tile_pool(name="wo", bufs=1) as wop,
                tc.tile_pool(name="ob", bufs=3) as obp,
                tc.tile_pool(name="ops", bufs=4, space="PSUM") as ops,
            ):
                wo_sb = wop.tile([128, 4, D], f32r)
                nc.sync.dma_start(
                    out=wo_sb, in_=wo_d.rearrange("(r p) n -> p r n", p=128))
                for qt in range(NKB):
                    ob = obp.tile([128, D], f32)
                    for ncol in range(2):
                        ps = ops.tile([128, 512], f32)
                        for p in range(PAIRS):
                            nc.tensor.matmul(
                                ps, yts[p][:, 128 * qt:128 * qt + 128],
                                wo_sb[:, p, 512 * ncol:512 * ncol + 512],
                                start=(p == 0), stop=(p == PAIRS - 1))
                        nc.vector.tensor_copy(
                            ob[:, 512 * ncol:512 * ncol + 512], ps)
                    nc.sync.dma_start(
                        out=out_d[128 * qt:128 * qt + 128, :], in_=ob)
    nc.finalize()
    return nc


def get_nc():
    if "nc" not in _CACHE:
        _CACHE["nc"] = _build_nc()
    return _CACHE["nc"]


def shard_inputs(x, w_qkv, b_qkv, w_o):
    """Per-core input dicts for cores 0..7."""
    x = np.asarray(x, dtype=np.float32)
    w_qkv = np.asarray(w_qkv, dtype=np.float32)
    b_qkv = np.asarray(b_qkv, dtype=np.float32)
    w_o = np.asarray(w_o, dtype=np.float32)
    xts = [np.ascontiguousarray(x[b].T) for b in range(x.shape[0])]
    in_maps = []
    for c in range(NCORES):
        b, g = divmod(c, 2)
        cols = slice(512 * g, 512 * g + 512)
        wq, wk, wv = (w_qkv[:, 1024 * i:1024 * (i + 1)][:, cols] for i in range(3))
        bq, bk, bv = (b_qkv[1024 * i:1024 * (i + 1)][cols] for i in range(3))
        in_maps.append({
            "xt": xts[b],
            "wqkv": np.ascontiguousarray(np.concatenate([wq, wk, wv], axis=1)),
            "bqkv": np.concatenate([bq, bk, bv])[None, :].copy(),
            "wo": np.ascontiguousarray(w_o[512 * g:512 * g + 512, :]),
        })
    return in_maps


def kernel(x, w_qkv, b_qkv, w_o, b_o):
    from concourse.bass_utils import run_bass_kernel_spmd

    nc = get_nc()
    in_maps = shard_inputs(x, w_qkv, b_qkv, w_o)
    res = run_bass_kernel_spmd(nc, in_maps, core_ids=list(range(NCORES)))
    parts = [r["out"] for r in res.results]
    b_o = np.asarray(b_o, dtype=np.float64)
    out = np.empty((4, SEQ, D), dtype=np.float32)
    for b in range(4):
        out[b] = (parts[2 * b].astype(np.float64)
                  + parts[2 * b + 1].astype(np.float64) + b_o).astype(np.float32)
    return out


# revision 9
# speedup vs baseline: 6954.3163x; 6954.3163x over previous
"""Causal self-attention (dense transformer block) on 8 Trainium2 NeuronCores.

Reference computation (per batch b of 4, 16 heads, head_dim 64, d=1024):
    qkv = x @ w_qkv + b_qkv ; q,k,v split
    att = softmax(causal(q k^T / 8)) ; y = att @ v ; out = y @ w_o + b_o

Sharding: batch (4) x head-halves (2) -> 8 cores. Core c handles batch c//2's
heads [8*(c%2), 8*(c%2)+8). Each core computes its QKV shard, causal attention
for its 8 heads, and a partial output projection over its 512 head-dims.
Host sums the two partials per batch and adds b_o.

On-core layout (all matmul operands float32r = full-rate fp32 on the PE):
  - x^T resident as [128, 8, 2048] per embed block (host pre-transposes).
  - Q^T, K^T per head-pair: [128, 2048] (head a on partitions 0:64, b on 64:128).
  - V per pair: [128(seq), kb, head, 65] with col 64 = 1.0 (softmax denominator
    rides along the att@V accumulation as output row 64).
  - scores computed transposed: s^T[k, q] = (K^T)^T Q^T, causally column-trimmed;
    triangular window masked by accumulating a -1e9 mask via identity matmul
    (keeps PSUM readers = ACT only).
  - exp on ScalarE with fused 1/8 scale; att@V accumulates y^T[65, q] per head;
    row 64 = sum -> reciprocal -> gpsimd partition-broadcast -> normalize.
  - output projection consumes y^T directly as lhsT.
"""

import numpy as np

D = 1024
SEQ = 2048
NH = 16
HD = 64
NCORES = 8
PAIRS = 4       # head-pairs per core (8 heads)
NQT = 4         # query tiles of 512
QT = 512
NKB = 16        # key blocks of 128
MASKVAL = -1e9

_CACHE = {}


def _build_nc(reps: int = 1):
    import concourse.mybir as mybir
    import concourse.tile as tile
    from concourse import bacc

    f32 = mybir.dt.float32
    f32r = mybir.dt.float32r
    EXP = mybir.ActivationFunctionType.Exp

    nc = bacc.Bacc("TRN2", target_bir_lowering=False, debug=False)
    xt_d = nc.dram_tensor("xt", [D, SEQ], f32r, kind="ExternalInput").ap()
    wqkv_d = nc.dram_tensor("wqkv", [D, 3 * 512], f32r, kind="ExternalInput").ap()
    bqkv_d = nc.dram_tensor("bqkv", [1, 3 * 512], f32r, kind="ExternalInput").ap()
    wo_d = nc.dram_tensor("wo", [512, D], f32r, kind="ExternalInput").ap()
    out_d = nc.dram_tensor("out", [SEQ, D], f32, kind="ExternalOutput").ap()

    with tile.TileContext(nc) as tc:
        import contextlib
        loop_ctx = tc.For_i(0, reps, 1) if reps > 1 else contextlib.nullcontext()
        with loop_ctx:
            _emit(nc, tc, mybir, xt_d, wqkv_d, bqkv_d, wo_d, out_d)
    nc.finalize()
    return nc


def _emit(nc, tc, mybir, xt_d, wqkv_d, bqkv_d, wo_d, out_d):
    f32 = mybir.dt.float32
    f32r = mybir.dt.float32r
    EXP = mybir.ActivationFunctionType.Exp
    if True:
        with (
            tc.tile_pool(name="const", bufs=1) as constp,
            tc.tile_pool(name="qt", bufs=PAIRS) as qtp,
            tc.tile_pool(name="kt", bufs=PAIRS) as ktp,
            tc.tile_pool(name="v", bufs=PAIRS) as vp,
            tc.tile_pool(name="yt", bufs=PAIRS) as ytp,
        ):
            # fp32r constants; built on fp32 scratch (memset/affine_select
            # are not fp32r-legal) then rounded in via tensor_copy
            ones = constp.tile([1, 512], f32r)
            ident = constp.tile([128, 128], f32r)
            mask = constp.tile([128, QT], f32r)
            zeros = constp.tile([128, 384], f32r)
            col1 = constp.tile([128, 1], f32r)
            with tc.tile_pool(name="scratch", bufs=1) as scrp:
                ones32 = scrp.tile([1, 512], f32)
                nc.gpsimd.memset(ones32, 1.0)
                nc.vector.tensor_copy(ones, ones32)
                ident32 = scrp.tile([128, 128], f32)
                nc.gpsimd.memset(ident32, 0.0)
                nc.gpsimd.affine_select(
                    out=ident32, in_=ident32,
                    compare_op=mybir.AluOpType.not_equal,
                    fill=1.0, base=0, pattern=[[-1, 128]], channel_multiplier=1)
                nc.vector.tensor_copy(ident, ident32)
                # mask[k, q] = 0 if k <= q else MASKVAL for q < 128; 0 beyond
                # (zero tail so a full-width accumulate closes the PSUM bank)
                mask32 = scrp.tile([128, QT], f32)
                nc.gpsimd.memset(mask32, 0.0)
                nc.gpsimd.affine_select(
                    out=mask32[:, 0:128], in_=mask32[:, 0:128],
                    compare_op=mybir.AluOpType.is_ge,
                    fill=MASKVAL, base=0, pattern=[[1, 128]],
                    channel_multiplier=-1)
                nc.vector.tensor_copy(mask, mask32)
                zeros32 = scrp.tile([128, 384], f32)
                nc.gpsimd.memset(zeros32, 0.0)
                nc.vector.tensor_copy(zeros, zeros32)
                ones_col32 = scrp.tile([128, 1], f32)
                nc.gpsimd.memset(ones_col32, 1.0)
                nc.vector.tensor_copy(col1, ones_col32)

            qts = [qtp.tile([128, SEQ], f32r, tag="qt", name=f"qt{i}") for i in range(PAIRS)]
            kts = [ktp.tile([128, SEQ], f32r, tag="kt", name=f"kt{i}") for i in range(PAIRS)]
            vs = [vp.tile([128, NKB, 2, 65], f32r, tag="v", name=f"v{i}") for i in range(PAIRS)]
            yts = [ytp.tile([128, SEQ], f32r, tag="yt", name=f"yt{i}") for i in range(PAIRS)]
            for p in range(PAIRS):
                nc.vector.tensor_copy(
                    vs[p][:, :, :, 64:65],
                    col1[:, None, None, :].broadcast_to([128, NKB, 2, 1]))

            # ---- Phase 1: QKV projections (x^T streamed by seq-tile) ----
            with (
                tc.tile_pool(name="xt", bufs=2) as xtp,
                tc.tile_pool(name="w1", bufs=1) as w1p,
                tc.tile_pool(name="ps1", bufs=4, space="PSUM") as ps1,
            ):
                wqkv_sb = w1p.tile([128, 8, 1536], f32r)
                nc.sync.dma_start(
                    out=wqkv_sb, in_=wqkv_d.rearrange("(e p) m -> p e m", p=128))
                bqkv_sb = w1p.tile([1, 1536], f32r)
                nc.sync.dma_start(out=bqkv_sb, in_=bqkv_d)

                ST = 256  # xt streaming tile width (seq)
                for s in range(SEQ // ST):
                    xt_s = xtp.tile([128, 8, ST], f32r)
                    nc.sync.dma_start(
                        out=xt_s,
                        in_=xt_d.rearrange("(e p) s -> p e s", p=128)[
                            :, :, s * ST:(s + 1) * ST])
                    # Q^T and K^T slices for each pair
                    for p in range(PAIRS):
                        for qk in range(2):
                            cols = 512 * qk + 128 * p
                            ps = ps1.tile([128, ST], f32)
                            nc.tensor.matmul(
                                ps, bqkv_sb[0:1, cols:cols + 128], ones[0:1, :ST],
                                start=True, stop=False)
                            for e in range(8):
                                nc.tensor.matmul(
                                    ps, wqkv_sb[:, e, cols:cols + 128],
                                    xt_s[:, e, :], start=False, stop=(e == 7))
                            dst = qts[p] if qk == 0 else kts[p]
                            nc.vector.tensor_copy(
                                dst[:, s * ST:(s + 1) * ST], ps)
                    # V rows for this seq-tile (all 8 heads at once)
                    for s4 in range(ST // 128):
                        kb = (ST // 128) * s + s4
                        ps = ps1.tile([128, 512], f32, tag="psv")
                        nc.tensor.matmul(
                            ps, ones[0:1, 0:128], bqkv_sb[0:1, 1024:1536],
                            start=True, stop=False)
                        for e in range(8):
                            nc.tensor.matmul(
                                ps, xt_s[:, e, 128 * s4:128 * s4 + 128],
                                wqkv_sb[:, e, 1024:1536],
                                start=False, stop=(e == 7))
                        for p in range(PAIRS):
                            for h in range(2):
                                nc.vector.tensor_copy(
                                    vs[p][:, kb, h, 0:64],
                                    ps[:, 128 * p + 64 * h:128 * p + 64 * h + 64])

            # ---- Phase 2: causal attention per pair / query-tile ----
            with (
                tc.tile_pool(name="pr", bufs=6) as prp,
                tc.tile_pool(name="sums", bufs=4) as sup,
                tc.tile_pool(name="rbc", bufs=4) as rbp,
                tc.tile_pool(name="scps", bufs=4, space="PSUM") as scps,
                tc.tile_pool(name="ytps", bufs=4, space="PSUM") as ytps,
            ):
                for p in range(PAIRS):
                    for j in range(NQT):
                        q0 = j * QT
                        nkb = 4 * j + 4
                        yt_ps = [ytps.tile([65, QT], f32, tag="ytps", name=f"ytps{h}") for h in range(2)]
                        for kb in range(nkb):
                            d = max(0, 128 * kb - q0)
                            for h in range(2):
                                hs = 64 * h
                                sc = scps.tile([128, QT], f32)
                                diag = 128 * kb >= q0
                                nc.tensor.matmul(
                                    sc[:, d:QT],
                                    kts[p][hs:hs + 64, 128 * kb:128 * kb + 128],
                                    qts[p][hs:hs + 64, q0 + d:q0 + QT],
                                    start=True, stop=not diag)
                                if diag:
                                    nc.tensor.matmul(
                                        sc[:, d:QT], ident, mask[:, 0:QT - d],
                                        start=False, stop=True)
                                pr = prp.tile([128, QT], f32r)
                                if d:
                                    # fully-masked columns: probs = 0 so the
                                    # att@V matmul can run full width
                                    nc.vector.tensor_copy(
                                        pr[:, 0:d], zeros[:, 0:d])
                                nc.scalar.activation(
                                    out=pr[:, d:QT], in_=sc[:, d:QT],
                                    func=EXP, scale=0.125)
                                nc.tensor.matmul(
                                    yt_ps[h], vs[p][:, kb, h, :],
                                    pr, start=(kb == 0),
                                    stop=(kb == nkb - 1))
                        for h in range(2):
                            rc = sup.tile([1, QT], mybir.dt.float32)
                            nc.vector.reciprocal(rc, yt_ps[h][64:65, :])
                            rb = rbp.tile([64, QT], mybir.dt.float32)
                            nc.gpsimd.partition_broadcast(rb, rc)
                            nc.vector.tensor_mul(
                                yts[p][64 * h:64 * h + 64, q0:q0 + QT],
                                yt_ps[h][0:64, :], rb)

            # ---- Phase 3: output projection (partial over 512 head-dims) ----
            with (
                tc.tile_pool(name="wo", bufs=1) as wop,
                tc.tile_pool(name="ob", bufs=3) as obp,
                tc.tile_pool(name="ops", bufs=4, space="PSUM") as ops,
            ):
                wo_sb = wop.tile([128, 4, D], f32r)
                nc.sync.dma_start(
                    out=wo_sb, in_=wo_d.rearrange("(r p) n -> p r n", p=128))
                for qt in range(NKB):
                    ob = obp.tile([128, D], f32)
                    for ncol in range(2):
                        ps = ops.tile([128, 512], f32)
                        for p in range(PAIRS):
                            nc.tensor.matmul(
                                ps, yts[p][:, 128 * qt:128 * qt + 128],
                                wo_sb[:, p, 512 * ncol:512 * ncol + 512],
                                start=(p == 0), stop=(p == PAIRS - 1))
                        nc.vector.tensor_copy(
                            ob[:, 512 * ncol:512 * ncol + 512], ps)
                    nc.sync.dma_start(
                        out=out_d[128 * qt:128 * qt + 128, :], in_=ob)


def get_nc(reps: int = 1):
    if reps not in _CACHE:
        _CACHE[reps] = _build_nc(reps)
    return _CACHE[reps]


def shard_inputs(x, w_qkv, b_qkv, w_o):
    """Per-core input dicts for cores 0..7."""
    x = np.asarray(x, dtype=np.float32)
    w_qkv = np.asarray(w_qkv, dtype=np.float32)
    b_qkv = np.asarray(b_qkv, dtype=np.float32)
    w_o = np.asarray(w_o, dtype=np.float32)
    xts = [np.ascontiguousarray(x[b].T) for b in range(x.shape[0])]
    in_maps = []
    for c in range(NCORES):
        b, g = divmod(c, 2)
        cols = slice(512 * g, 512 * g + 512)
        wq, wk, wv = (w_qkv[:, 1024 * i:1024 * (i + 1)][:, cols] for i in range(3))
        bq, bk, bv = (b_qkv[1024 * i:1024 * (i + 1)][cols] for i in range(3))
        in_maps.append({
            "xt": xts[b],
            "wqkv": np.ascontiguousarray(np.concatenate([wq, wk, wv], axis=1)),
            "bqkv": np.concatenate([bq, bk, bv])[None, :].copy(),
            "wo": np.ascontiguousarray(w_o[512 * g:512 * g + 512, :]),
        })
    return in_maps


def kernel(x, w_qkv, b_qkv, w_o, b_o):
    from concourse.bass_utils import run_bass_kernel_spmd

    nc = get_nc()
    in_maps = shard_inputs(x, w_qkv, b_qkv, w_o)
    res = run_bass_kernel_spmd(nc, in_maps, core_ids=list(range(NCORES)))
    parts = [r["out"] for r in res.results]
    b_o = np.asarray(b_o, dtype=np.float64)
    out = np.empty((4, SEQ, D), dtype=np.float32)
    for b in range(4):
        out[b] = (parts[2 * b].astype(np.float64)
                  + parts[2 * b + 1].astype(np.float64) + b_o).astype(np.float32)
    return out
